# revision 1
# baseline (speedup 1.0000x reference)
"""Dilated LSTM (B=8, T=256, C=1024, H=2048, dilation=4) on 8 trn2 NeuronCores.

Strategy
--------
dilation=4 makes timesteps t and t-4 adjacent in the recurrence, so the
sequence splits into 4 independent chains; batching them gives 64 supersteps
over an effective batch of NSEQ = B*D = 32 sequences.

w_hh is 67MB fp32 (doesn't fit one core's SBUF), so the 4H gate dimension is
split 8 ways (tensor parallel).  Core k owns a 1024-row slice of w_ih/w_hh
(gate-chunk order [i, f, o, g], h-dims [k*256,(k+1)*256)), kept resident in
SBUF transposed.  Each superstep:
  - PSUM u[128,256] accumulates x-projection (8 K-tiles) + h-recurrence
    (16 K-tiles), 4 column-tiled matmuls per K-tile.  Column group j owns
    out partitions 32j..32j+32 and computes ALL FOUR gates for h-dim
    quarter j of the core's 256-dim slice; the free dim is [i|f|o|g]x64.
    So every elementwise op spans all 128 partitions at base 0 (walrus
    requires equal base partitions for SBUF tensor-tensor ops).
  - gates: sigmoid on free cols 0..192 (i,f,o), tanh on 192..256 (g);
    c/h updates on [128,64] tiles.
  - h_new [128,64] is 32x32-block-transposed (DVE) and DMA'd to a DRAM
    bounce tile [128,64]; an 8-core AllGather concatenates to [1024,64],
    which is DMA'd back into the hT stationary buffer (pure-permutation
    access pattern) for the next step.

Host side transposes/permutes inputs (free), shards weights, and reassembles
the output.
"""

import numpy as np

B, T, C, H, D = 8, 256, 1024, 2048, 4
NCORES = 8
SLICE = H // NCORES      # 256 h-dims owned per core
TS = T // D              # 64 supersteps
NSEQ = B * D             # 32 sequences
KT_C = C // 128          # 8  K-tiles for the input projection
KT_H = H // 128          # 16 K-tiles for the recurrence

# bf16 matmul operands (fp32 PSUM accumulation, fp32 gates/state/output).
# fp32 matmuls stream at 1/4 rate on trn2 ("2 half-speed matmuls" in the
# cost model), so this is ~4x PE throughput; the h-exchange also halves.
MM_BF16 = True

_CACHE = {}


def _build_nc():
    import concourse.bass as bass
    import concourse.mybir as mybir
    import concourse.tile as tile
    from concourse import bacc

    f32 = mybir.dt.float32
    fmm = mybir.dt.bfloat16 if MM_BF16 else f32
    AF = mybir.ActivationFunctionType

    nc = bacc.Bacc(
        "TRN2",
        target_bir_lowering=False,
        debug=False,
        enable_asserts=False,
        num_devices=NCORES,
    )

    xT = nc.dram_tensor("xT", [KT_C, 128, TS * NSEQ], fmm, kind="ExternalInput")
    wihT = nc.dram_tensor("wihT", [C, 4 * SLICE], fmm, kind="ExternalInput")
    whhT = nc.dram_tensor("whhT", [H, 4 * SLICE], fmm, kind="ExternalInput")
    bias4 = nc.dram_tensor("bias4", [4, SLICE], f32, kind="ExternalInput")
    ind4 = nc.dram_tensor("ind4", [4, 128], f32, kind="ExternalInput")
    out_d = nc.dram_tensor("out", [TS, 128, SLICE // 4], f32, kind="ExternalOutput")

    with tile.TileContext(nc) as tc:
        with (
            tc.tile_pool(name="const", bufs=1) as const,
            tc.tile_pool(name="state", bufs=1) as state,
            tc.tile_pool(name="work", bufs=3) as work,
            tc.tile_pool(name="psum", bufs=4, space="PSUM") as psum,
            tc.tile_pool(name="dram", bufs=2, space="DRAM") as dram,
        ):
            # --- resident tensors -----------------------------------------
            x_sb = const.tile([128, KT_C * TS * NSEQ], fmm)
            wih_sb = const.tile([128, KT_C * 4 * SLICE], fmm)
            whh_sb = const.tile([128, KT_H * 4 * SLICE], fmm)
            bias_sb = const.tile([4, SLICE], f32)
            ind_sb = const.tile([4, 128], f32)
            nc.sync.dma_start(ind_sb[:], ind4[:])
            for t in range(KT_C):
                nc.sync.dma_start(
                    x_sb[:, t * (TS * NSEQ):(t + 1) * (TS * NSEQ)], xT[t]
                )
                nc.sync.dma_start(
                    wih_sb[:, t * (4 * SLICE):(t + 1) * (4 * SLICE)],
                    wihT[t * 128:(t + 1) * 128, :],
                )
            for t in range(KT_H):
                nc.sync.dma_start(
                    whh_sb[:, t * (4 * SLICE):(t + 1) * (4 * SLICE)],
                    whhT[t * 128:(t + 1) * 128, :],
                )
            nc.sync.dma_start(bias_sb[:], bias4[:])

            # --- recurrent state ------------------------------------------
            hT_sb = state.tile([128, KT_H * NSEQ], fmm)  # h^T, K-tile t at cols 32t
            c_sb = state.tile([128, SLICE // 4], f32)    # c, (quarter,seq) x 64
            nc.gpsimd.memset(hT_sb[:], 0.0)
            nc.gpsimd.memset(c_sb[:], 0.0)

            for s in range(TS):
                ps = psum.tile([128, SLICE], f32, name=f"ps{s}", tag="ps")
                # init: u = bias (per gate chunk), one full-region matmul
                nc.tensor.matmul(
                    ps[:], ind_sb[:], bias_sb[:],
                    start=True, stop=False, skip_group_check=True,
                )
                # projection: no dependence on the gather -> overlap filler
                for t in range(KT_C):
                    lhs = x_sb[:, t * (TS * NSEQ) + s * NSEQ:
                               t * (TS * NSEQ) + (s + 1) * NSEQ]
                    for j in range(4):
                        nc.tensor.matmul(
                            ps[32 * j:32 * (j + 1), :],
                            lhs,
                            wih_sb[:, t * 4 * SLICE + j * SLICE:
                                   t * 4 * SLICE + (j + 1) * SLICE],
                            start=False,
                            stop=False,
                            tile_position=(0, 32 * j),
                            skip_group_check=True,
                        )
                # recurrence: waits on hT gather of the previous step
                for t in range(KT_H):
                    lhs = hT_sb[:, t * NSEQ:(t + 1) * NSEQ]
                    for j in range(4):
                        nc.tensor.matmul(
                            ps[32 * j:32 * (j + 1), :],
                            lhs,
                            whh_sb[:, t * 4 * SLICE + j * SLICE:
                                   t * 4 * SLICE + (j + 1) * SLICE],
                            start=False,
                            stop=(t == KT_H - 1),
                            tile_position=(0, 32 * j),
                            skip_group_check=True,
                        )

                # gates.  partition 32j+m = (h-quarter j, seq m);
                # free cols: 0..64 = i, 64..128 = f, 128..192 = o, 192..256 = g
                Q = SLICE // 4  # 64
                sig = work.tile([128, 3 * Q], f32, name=f"sig{s}", tag="sig")
                nc.scalar.activation(sig[:], ps[:, 0:3 * Q], AF.Sigmoid)
                tg = work.tile([128, Q], f32, name=f"tg{s}", tag="tg")
                nc.scalar.activation(tg[:], ps[:, 3 * Q:4 * Q], AF.Tanh)
                t1 = work.tile([128, Q], f32, name=f"t1{s}", tag="t1")
                nc.vector.tensor_mul(t1[:], sig[:, 0:Q], tg[:])
                nc.vector.tensor_mul(c_sb[:], sig[:, Q:2 * Q], c_sb[:])
                nc.vector.tensor_add(c_sb[:], c_sb[:], t1[:])
                tct = work.tile([128, Q], f32, name=f"tct{s}", tag="tct")
                nc.scalar.activation(tct[:], c_sb[:], AF.Tanh)
                h_sb = work.tile([128, Q], f32, name=f"h{s}", tag="h")
                nc.vector.tensor_mul(h_sb[:], sig[:, 2 * Q:3 * Q], tct[:])

                nc.sync.dma_start(out_d[s], h_sb[:])

                # h_new -> blockwise transpose -> AllGather -> hT_sb
                # bt[32j+p, 32b+m] = h_sb[32j+m, 32b+p]
                #                  = h^T[local dim 64j+32b+p, seq m]
                if MM_BF16:
                    h_mm = work.tile([128, Q], fmm, name=f"hb{s}", tag="hb")
                    nc.vector.tensor_copy(h_mm[:], h_sb[:])
                else:
                    h_mm = h_sb
                bt = work.tile([128, Q], fmm, name=f"bt{s}", tag="bt")
                nc.vector.transpose(bt[:], h_mm[:])
                cc_in = dram.tile([128, Q], fmm, name=f"cci{s}", tag="cci")
                nc.sync.dma_start(cc_in[:], bt[:])
                cc_out = dram.tile(
                    [NCORES * 128, Q], fmm, name=f"cco{s}", tag="cco",
                    addr_space="Shared",
                )
                nc.gpsimd.collective_compute(
                    "AllGather",
                    mybir.AluOpType.bypass,
                    replica_groups=[list(range(NCORES))],
                    ins=[cc_in[:]],
                    outs=[cc_out[:]],
                )
                # cc_out row 128k+32j+p, col 32b+m holds h^T[d, m] with
                # d = 256k+64j+32b+p.  Writing kj = 2k+(j>>1), jl = j&1:
                # row = 64*kj + 32*jl + p, and the dest is
                # hT_sb[64*jl+32*b+p, 32*kj+m].  Split by (jl, b) so each
                # DMA is a 3-dim access pattern.
                cco_v = cc_out[:].rearrange(
                    "(kj jl p) (b m) -> jl b p kj m", jl=2, p=32, b=2
                )
                for jl in range(2):
                    for b in range(2):
                        nc.sync.dma_start(
                            hT_sb[64 * jl + 32 * b:64 * jl + 32 * b + 32, :]
                            .rearrange("p (kj m) -> p kj m", m=NSEQ),
                            cco_v[jl, b],
                        )

    nc.compile()
    return nc


def _host_inputs(x, w_ih, b_ih, w_hh, b_hh):
    x = np.ascontiguousarray(np.asarray(x, dtype=np.float32))
    w_ih = np.asarray(w_ih, dtype=np.float32)
    b_ih = np.asarray(b_ih, dtype=np.float32)
    w_hh = np.asarray(w_hh, dtype=np.float32)
    b_hh = np.asarray(b_hh, dtype=np.float32)

    if MM_BF16:
        import ml_dtypes
        mm_np = ml_dtypes.bfloat16
    else:
        mm_np = np.float32

    # x -> [K-tile, partition, (s, b, c)] with columns ordered s*32 + b*4 + c
    xr = x.reshape(B, TS, D, KT_C, 128)          # b, s, c, t, p
    xr = np.ascontiguousarray(xr.transpose(3, 4, 1, 0, 2))  # t, p, s, b, c
    xT = xr.reshape(KT_C, 128, TS * NSEQ).astype(mm_np)

    bias = b_ih + b_hh
    ind4 = np.zeros((4, 128), dtype=np.float32)
    for j in range(4):
        ind4[j, 32 * j:32 * (j + 1)] = 1.0
    # rhs column order per core: (quarter j, gate g in [i,f,o,g], n in 0..64)
    # -> global weight row goff[g] + k*SLICE + 64*j + n
    goff = np.array([0, H, 3 * H, 2 * H])  # reference gate order (i, f, g, o)
    Q = SLICE // 4
    jj, gg, nn = np.meshgrid(
        np.arange(4), np.arange(4), np.arange(Q), indexing="ij"
    )
    in_maps = []
    for k in range(NCORES):
        idx = (goff[gg] + k * SLICE + Q * jj + nn).reshape(-1)
        wihT = np.ascontiguousarray(w_ih[idx, :].T).astype(mm_np)  # [C, 1024]
        whhT = np.ascontiguousarray(w_hh[idx, :].T).astype(mm_np)  # [H, 1024]
        bias4 = np.ascontiguousarray(bias[idx].reshape(4, SLICE))
        in_maps.append({
            "xT": xT, "wihT": wihT, "whhT": whhT,
            "bias4": bias4, "ind4": ind4,
        })
    return in_maps


def kernel(x, w_ih, b_ih, w_hh, b_hh, dilation):
    from concourse.bass_utils import run_bass_kernel_spmd

    assert int(dilation) == D, f"kernel hardcodes dilation={D}, got {dilation}"
    assert tuple(np.shape(x)) == (B, T, C)

    if "nc" not in _CACHE:
        _CACHE["nc"] = _build_nc()
    nc = _CACHE["nc"]

    in_maps = _host_inputs(x, w_ih, b_ih, w_hh, b_hh)

    import time

    t0 = time.perf_counter()
    res = run_bass_kernel_spmd(nc, in_maps, core_ids=list(range(NCORES)))
    _CACHE["last_wall_s"] = time.perf_counter() - t0
    _CACHE["last_exec_ns"] = res.exec_time_ns

    return _assemble([r["out"] for r in res.results])


def _assemble(outs):
    # out_k[s, 32j + (b*4+c), n] -> full[b, s*4+c, k*256+64j+n]
    o = np.stack(outs)                              # [8, 64, 128, 64]
    o = o.reshape(NCORES, TS, 4, B, D, SLICE // 4)  # k, s, j, b, c, n
    o = o.transpose(3, 1, 4, 0, 2, 5)               # b, s, c, k, j, n
    return np.ascontiguousarray(o.reshape(B, T, H), dtype=np.float32)



# revision 2
# speedup vs baseline: 43.1415x; 43.1415x over previous
"""Dilated LSTM (B=8, T=256, C=1024, H=2048, dilation=4) on 8 trn2 NeuronCores.

Strategy (v3: collective exchange, streamlined layout)
------------------------------------------------------
dilation=4 splits the sequence into 4 independent chains; batching them gives
64 supersteps over NSEQ = B*D = 32 sequences.  The 4H gate dimension is split
8 ways (tensor parallel); each superstep the full h vector is rebuilt on every
core with an ncfw AllGather (~6us mesh; measured cheaper than any
remote_dma_broadcast pattern, whose dummy-descriptor tax costs ~34us/step).

Layout improvements over the v1 baseline:
- gate column dim mapping d = 128*b + 32*j + p: the DVE 32x32 block transpose
  of the bf16 gate tile lands the core's h-chunk DIRECTLY in hT(k-tile, seq)
  layout — it doubles as the AllGather input staging tile (no fp32->bf16 CAST,
  no repack).
- AllGather concatenates chunks in plain core order, so w_hh keeps its
  natural contraction order and the gather-out -> hT SBUF load is ONE
  contiguous-destination DMA (`(k p) c -> p (k c)`) instead of 4 permuted ones.
- the per-step output tile is the bf16 hT chunk itself (host upcasts).
"""

import numpy as np

B, T, C, H, D = 8, 256, 1024, 2048, 4
NCORES = 8
SLICE = H // NCORES      # 256 h-dims owned per core
TS = T // D              # 64 supersteps
NSEQ = B * D             # 32 sequences
KT_C = C // 128          # 8  K-tiles for the input projection
KT_H = H // 128          # 16 K-tiles for the recurrence
Q = SLICE // 4           # 64

_CACHE = {}


def _build_nc():
    import concourse.mybir as mybir
    import concourse.tile as tile
    from concourse import bacc

    f32 = mybir.dt.float32
    bf16 = mybir.dt.bfloat16
    AF = mybir.ActivationFunctionType

    nc = bacc.Bacc(
        "TRN2",
        target_bir_lowering=False,
        debug=False,
        enable_asserts=False,
        num_devices=NCORES,
    )

    xT = nc.dram_tensor("xT", [KT_C, 128, TS * NSEQ], bf16, kind="ExternalInput")
    wihT = nc.dram_tensor("wihT", [C, 4 * SLICE], bf16, kind="ExternalInput")
    whhT = nc.dram_tensor("whhT", [H, 4 * SLICE], bf16, kind="ExternalInput")
    bias4 = nc.dram_tensor("bias4", [4, SLICE], f32, kind="ExternalInput")
    ind4 = nc.dram_tensor("ind4", [4, 128], f32, kind="ExternalInput")
    out_d = nc.dram_tensor("out", [TS, 128, Q], bf16, kind="ExternalOutput")

    with tile.TileContext(nc) as tc:
        with (
            tc.tile_pool(name="const", bufs=1) as const,
            tc.tile_pool(name="state", bufs=1) as state,
            tc.tile_pool(name="work", bufs=3) as work,
            tc.tile_pool(name="psum", bufs=4, space="PSUM") as psum,
            tc.tile_pool(name="dram", bufs=2, space="DRAM") as dram,
        ):
            # --- resident tensors -----------------------------------------
            x_sb = const.tile([128, KT_C * TS * NSEQ], bf16)
            wih_sb = const.tile([128, KT_C * 4 * SLICE], bf16)
            whh_sb = const.tile([128, KT_H * 4 * SLICE], bf16)
            bias_sb = const.tile([4, SLICE], f32)
            ind_sb = const.tile([4, 128], f32)
            nc.sync.dma_start(ind_sb[:], ind4[:])
            for t in range(KT_C):
                nc.sync.dma_start(
                    x_sb[:, t * (TS * NSEQ):(t + 1) * (TS * NSEQ)], xT[t]
                )
                nc.sync.dma_start(
                    wih_sb[:, t * (4 * SLICE):(t + 1) * (4 * SLICE)],
                    wihT[t * 128:(t + 1) * 128, :],
                )
            for t in range(KT_H):
                nc.sync.dma_start(
                    whh_sb[:, t * (4 * SLICE):(t + 1) * (4 * SLICE)],
                    whhT[t * 128:(t + 1) * 128, :],
                )
            nc.sync.dma_start(bias_sb[:], bias4[:])

            # --- recurrent state ------------------------------------------
            # hT: k-tile t at cols [32t, 32t+32); k-tiles 2k,2k+1 = core k.
            hT_sb = state.tile([128, KT_H * NSEQ], bf16)
            c_sb = state.tile([128, Q], f32)
            nc.gpsimd.memset(hT_sb[:], 0.0)
            nc.gpsimd.memset(c_sb[:], 0.0)

            for s in range(TS):
                ps = psum.tile([128, SLICE], f32, name=f"ps{s}", tag="ps")
                # u = bias (per gate chunk), one full-region matmul
                nc.tensor.matmul(
                    ps[:], ind_sb[:], bias_sb[:],
                    start=True, stop=False, skip_group_check=True,
                )
                # projection: no dependence on the gather -> overlap filler
                for t in range(KT_C):
                    lhs = x_sb[:, t * (TS * NSEQ) + s * NSEQ:
                               t * (TS * NSEQ) + (s + 1) * NSEQ]
                    for j in range(4):
                        nc.tensor.matmul(
                            ps[32 * j:32 * (j + 1), :],
                            lhs,
                            wih_sb[:, t * 4 * SLICE + j * SLICE:
                                   t * 4 * SLICE + (j + 1) * SLICE],
                            start=False,
                            stop=False,
                            tile_position=(0, 32 * j),
                            skip_group_check=True,
                        )
                # recurrence: waits on hT gather of the previous step
                for t in range(KT_H):
                    lhs = hT_sb[:, t * NSEQ:(t + 1) * NSEQ]
                    for j in range(4):
                        nc.tensor.matmul(
                            ps[32 * j:32 * (j + 1), :],
                            lhs,
                            whh_sb[:, t * 4 * SLICE + j * SLICE:
                                   t * 4 * SLICE + (j + 1) * SLICE],
                            start=False,
                            stop=(t == KT_H - 1),
                            tile_position=(0, 32 * j),
                            skip_group_check=True,
                        )

                # gates.  partition 32j+m = (quarter j, seq m);
                # free cols: 0..64 = i, 64..128 = f, 128..192 = o, 192..256 = g
                sig = work.tile([128, 3 * Q], f32, name=f"sig{s}", tag="sig")
                nc.scalar.activation(sig[:], ps[:, 0:3 * Q], AF.Sigmoid)
                tg = work.tile([128, Q], f32, name=f"tg{s}", tag="tg")
                nc.scalar.activation(tg[:], ps[:, 3 * Q:4 * Q], AF.Tanh)
                t1 = work.tile([128, Q], f32, name=f"t1{s}", tag="t1")
                nc.vector.tensor_mul(t1[:], sig[:, 0:Q], tg[:])
                nc.vector.tensor_mul(c_sb[:], sig[:, Q:2 * Q], c_sb[:])
                nc.vector.tensor_add(c_sb[:], c_sb[:], t1[:])
                tct = work.tile([128, Q], f32, name=f"tct{s}", tag="tct")
                nc.scalar.activation(tct[:], c_sb[:], AF.Tanh)
                h_mm = work.tile([128, Q], bf16, name=f"hb{s}", tag="hb")
                nc.vector.tensor_mul(h_mm[:], sig[:, 2 * Q:3 * Q], tct[:])

                # block-transpose lands the own chunk in hT(k-tile,seq) layout
                own = work.tile([128, Q], bf16, name=f"own{s}", tag="own")
                nc.vector.transpose(own[:], h_mm[:])
                nc.sync.dma_start(out_d[s], own[:])

                # AllGather in plain core order; single contiguous load back.
                cc_in = dram.tile([128, Q], bf16, name=f"cci{s}", tag="cci")
                nc.sync.dma_start(cc_in[:], own[:])
                cc_out = dram.tile(
                    [NCORES * 128, Q], bf16, name=f"cco{s}", tag="cco",
                    addr_space="Shared",
                )
                nc.gpsimd.collective_compute(
                    "AllGather",
                    mybir.AluOpType.bypass,
                    replica_groups=[list(range(NCORES))],
                    ins=[cc_in[:]],
                    outs=[cc_out[:]],
                )
                nc.sync.dma_start(
                    hT_sb[:].rearrange("p (k c) -> p k c", c=Q),
                    cc_out[:].rearrange("(k p) c -> p k c", p=128),
                )

    nc.compile()
    return nc


def _host_inputs(x, w_ih, b_ih, w_hh, b_hh):
    import ml_dtypes

    x = np.ascontiguousarray(np.asarray(x, dtype=np.float32))
    w_ih = np.asarray(w_ih, dtype=np.float32)
    b_ih = np.asarray(b_ih, dtype=np.float32)
    w_hh = np.asarray(w_hh, dtype=np.float32)
    b_hh = np.asarray(b_hh, dtype=np.float32)
    mm_np = ml_dtypes.bfloat16

    # x -> [K-tile, partition, (s, b, c)] with columns ordered s*32 + b*4 + c
    xr = x.reshape(B, TS, D, KT_C, 128)                     # b, s, c, t, p
    xr = np.ascontiguousarray(xr.transpose(3, 4, 1, 0, 2))  # t, p, s, b, c
    xT = xr.reshape(KT_C, 128, TS * NSEQ).astype(mm_np)

    bias = b_ih + b_hh
    ind4 = np.zeros((4, 128), dtype=np.float32)
    for j in range(4):
        ind4[j, 32 * j:32 * (j + 1)] = 1.0

    # gate column order per core: (quarter j, gate g in [i,f,o,g], n=32b+p)
    # -> local dim d = 128b + 32j + p, weight row goff[g] + 256*core + d
    goff = np.array([0, H, 3 * H, 2 * H])  # reference gate order (i, f, g, o)
    jj, gg, nn = np.meshgrid(
        np.arange(4), np.arange(4), np.arange(Q), indexing="ij"
    )
    bb, pp = nn // 32, nn % 32
    dloc = 128 * bb + 32 * jj + pp          # [4,4,64] local dim per column

    # contraction (K) row order: k-tile kt=2k+hb, partition p ->
    # global dim 256*k + 128*hb + p  (plain core order)
    kt = np.arange(H) // 128
    p_r = np.arange(H) % 128
    k_r, hb_r = kt // 2, kt % 2
    krows = 256 * k_r + 128 * hb_r + p_r

    in_maps = []
    for k in range(NCORES):
        col_rows = (goff[gg] + k * SLICE + dloc).reshape(-1)
        wih_k = np.ascontiguousarray(w_ih[col_rows, :].T).astype(mm_np)
        whh_k = np.ascontiguousarray(w_hh[col_rows, :][:, krows].T).astype(mm_np)
        bias_k = np.ascontiguousarray(bias[col_rows].reshape(4, SLICE))
        in_maps.append({
            "xT": xT, "wihT": wih_k, "whhT": whh_k,
            "bias4": bias_k, "ind4": ind4,
        })
    return in_maps


def _assemble(outs):
    # out_k[s, part, 32b+m] = h(seq m, global dim 256k + 128b + part)
    o = np.stack([np.asarray(ok, dtype=np.float32) for ok in outs])
    o = o.reshape(NCORES, TS, 128, 2, B, D)     # k, s, part, b, batch, chain
    o = o.transpose(4, 1, 5, 0, 3, 2)           # batch, s, chain, k, b, part
    return np.ascontiguousarray(o.reshape(B, T, H), dtype=np.float32)


def kernel(x, w_ih, b_ih, w_hh, b_hh, dilation):
    from concourse.bass_utils import run_bass_kernel_spmd

    assert int(dilation) == D, f"kernel hardcodes dilation={D}, got {dilation}"
    assert tuple(np.shape(x)) == (B, T, C)

    if "nc" not in _CACHE:
        _CACHE["nc"] = _build_nc()
    nc = _CACHE["nc"]

    in_maps = _host_inputs(x, w_ih, b_ih, w_hh, b_hh)

    import time

    t0 = time.perf_counter()
    res = run_bass_kernel_spmd(nc, in_maps, core_ids=list(range(NCORES)))
    _CACHE["last_wall_s"] = time.perf_counter() - t0
    _CACHE["last_exec_ns"] = res.exec_time_ns

    return _assemble([r["out"] for r in res.results])
